# revision 16
# baseline (speedup 1.0000x reference)
"""Trainium2 Bass kernel for nn_Decoder: dense MLP + fixed-COO sparse matmul.

Computation:
    h = sigmoid(w @ W1.T + b1)                       # [B=128, H=8192]
    out_sp[b, r] = sum_e{rows[e]==r} edge_vals[e] * h[b, cols[e]]
    out = scale * out_sp + ref                       # [B, OUT=32768]

Strategy (8 NeuronCores, SPMD, row-partitioned, fp8):
  - Core k owns output rows [4096k, 4096(k+1)); the COO slice is
    canonicalized host-side to a dense per-core S[h, r] (duplicates
    summed) and streamed from HBM as e4m3 — 32 MB/core, the kernel's
    dominant (memory-bound) term at ~358 GB/s.
  - Centered activation for fp8 accuracy: with h = (1 + tanh(z/2))/2,
        scale*out_sp = tanh(z/2) @ (0.5*scale*S) + 0.5*scale*colsum(S)
    so the streamed matrix is S'' = 0.5*scale*S (e4m3) and the moving
    operand h'' = tanh(z/2) is zero-mean (std 0.42 vs sigmoid's 0.54
    rms), cutting both quantization error terms ~2.6x. The exact
    0.5*scale*colsum + ref correction folds into a [1, 4096] fp16 row
    injected into each PSUM bank via a K=1 ones matmul (start=True).
  - Stage B matmuls run perf_mode=DoubleRow (fp8-only, 2 K-tiles per
    pass, 0.5 cycles/row at FD=512) keeping PE well under the DMA rate.
  - Stage A (z = w @ W1.T) runs in e4m3 too (plain matmuls at bf16
    speed) so W1 streams at 2 MB; b1 is applied as a per-partition ACT
    bias (0.5*b1, since tanh(z/2 + b1/2)), and ACT writes fp8 h''
    directly. Measured end-to-end rel err ~1.1e-2 vs the 2e-2 gate.
  - Output written fp16 (host upcasts), halving writeback traffic.
"""

import numpy as np
import ml_dtypes

import concourse.bass as bass
import concourse.mybir as mybir
from concourse.tile import TileContext
from concourse.bass_utils import run_bass_kernel_spmd

LATENT, HIDDEN, OUT, BATCH = 256, 8192, 32768, 128
NCORES = 8
RPC = OUT // NCORES          # rows per core = 4096
RBLK = 512                   # output rows per PSUM bank
NRB = RPC // RBLK            # 8 row blocks per core
NPASS = 2
HRB = NRB // NPASS           # row blocks per pass = 4
HB = HIDDEN // 128           # 64 hidden chunks
CCP = HB // 2                # 32 hidden chunk-pairs (DoubleRow K-tiles)

_NC_CACHE = {}


def _split_multiwaits(nc):
    """walrus codegen embeds at most ONE sync wait per ISA instruction and
    errors with "Too many sync wait commands" otherwise.  Split extra waits
    into single-wait NoOps on the same engine immediately before the
    instruction (engine streams keep program order through walrus)."""
    for f in nc.m.functions:
        for bb in f.blocks:
            out, changed = [], False
            for ins in bb.instructions:
                si = ins.sync_info
                waits = list(si.on_wait) if si and si.on_wait else []
                if len(waits) > 1:
                    changed = True
                    for wsub in waits[:-1]:
                        n = mybir.InstNoOp(name=f"I-{nc.next_id()}", ins=[], outs=[])
                        n.engine = ins.engine
                        n.sync_info = mybir.SyncInfo(on_wait=[wsub], on_update=[])
                        out.append(n)
                    ins.sync_info = mybir.SyncInfo(
                        on_wait=waits[-1:], on_update=list(si.on_update or [])
                    )
                out.append(ins)
            if changed:
                bb.instructions = out


def _build_nc():
    fp32 = mybir.dt.float32
    f16 = mybir.dt.float16
    f8 = mybir.dt.float8e4
    TANH = mybir.ActivationFunctionType.Tanh
    DROW = mybir.MatmulPerfMode.DoubleRow

    nc = bass.Bass("TRN2", target_bir_lowering=False, debug=False)

    HBL = HB // NCORES       # hidden chunks computed locally per core = 8

    d_w1t = nc.dram_tensor("w1t8", [LATENT, HBL * 128], f8, kind="ExternalInput")
    d_wt = nc.dram_tensor("wt8", [LATENT, BATCH], f8, kind="ExternalInput")
    d_b1c = nc.dram_tensor("b1c", [128, HBL], fp32, kind="ExternalInput")
    d_hx = nc.dram_tensor("hx", [128 * HBL * BATCH], f8, kind="Internal")
    d_hxg = nc.dram_tensor(
        "hxg", [NCORES * 128 * HBL * BATCH], f8, kind="Internal", addr_space="Shared"
    )
    d_s = nc.dram_tensor(
        "s8", [NPASS, CCP // 2, 128, 2, HRB, 2, RBLK], f8, kind="ExternalInput"
    )
    d_rf = nc.dram_tensor("refb", [128, RPC], f16, kind="ExternalInput")
    d_out = nc.dram_tensor("out", [BATCH, RPC], f16, kind="ExternalOutput")

    with TileContext(nc) as tc:
        with (
            tc.tile_pool(name="consts", bufs=1) as consts,
            # 12 x 8KB/partition (2 ccpairs per slab) of S prefetch keeps the
            # DMA rings streaming through stage A and PSUM turnarounds, while
            # holding the semaphore count down (teardown zeroes each one).
            tc.tile_pool(name="sstream", bufs=12) as sstream,
            tc.tile_pool(name="work", bufs=2) as work,
            tc.tile_pool(name="wstream", bufs=8) as wstream,
        ):
            # ------------- constant + W1 loads, ahead of the S stream -------------
            # Everything stage A needs goes into the in-order SP FIFO before
            # the first S slab so the h'' pipeline is never stuck behind
            # 24 buffered slab transfers.
            sb_wt = consts.tile([128, 2, BATCH], f8)
            nc.sync.dma_start(
                out=sb_wt[:],
                in_=d_wt.ap().rearrange("(kc p) b -> p kc b", p=128),
            )
            w1t_ap = d_w1t.ap().rearrange("(kc p) h -> p kc h", p=128)
            wq = wstream.tile([128, 2, HBL * 128], f8, tag="wq")
            nc.sync.dma_start(out=wq[:], in_=w1t_ap)
            sb_b1 = consts.tile([128, HBL], fp32)
            nc.gpsimd.dma_start(out=sb_b1[:], in_=d_b1c.ap())
            sb_rf = consts.tile([128, RPC], f16)
            nc.gpsimd.dma_start(out=sb_rf[:], in_=d_rf.ap())

            htloc = consts.tile([128, HBL, BATCH], f8)
            ht_sb = consts.tile([128, HB, BATCH], f8)
            obuf = consts.tile([128, NRB, RBLK], f16)

            # ------- stage A (sharded): h'' = tanh(z/2) for 8 local chunks -------
            # Core k computes hidden chunks [8k, 8k+8) only (its w1t8/b1c
            # inputs are the per-core slices), then an 8-core DRAM AllGather
            # assembles the full ht in canonical core order.
            with tc.tile_pool(name="psA", bufs=2, space="PSUM") as psA:
                for quad in range(HBL // 4):
                    ps = psA.tile([128, 512], fp32, tag="hps")
                    for i4 in range(4):
                        i = quad * 4 + i4
                        nc.tensor.matmul(
                            ps[:, i4 * 128 : (i4 + 1) * 128],
                            lhsT=wq[:, :, i * 128 : (i + 1) * 128],
                            rhs=sb_wt[:],
                            start=True,
                            stop=True,
                            perf_mode=DROW,
                        )
                    for i4 in range(4):
                        i = quad * 4 + i4
                        nc.scalar.activation(
                            htloc[:, i, :],
                            ps[:, i4 * 128 : (i4 + 1) * 128],
                            TANH,
                            bias=sb_b1[:, i : i + 1],
                            scale=0.5,
                        )

            # Exchange: SBUF -> local DRAM -> AllGather -> full ht in SBUF.
            # Both transfers ride the scalar HWDGE queue so the in-order SP
            # FIFO keeps streaming S slabs behind them.
            nc.scalar.dma_start(
                out=d_hx.ap().rearrange("(p cl b) -> p cl b", p=128, cl=HBL),
                in_=htloc[:],
            )
            nc.gpsimd.collective_compute(
                kind="AllGather",
                op=mybir.AluOpType.bypass,
                replica_groups=[list(range(NCORES))],
                ins=[d_hx.ap()],
                outs=[d_hxg.ap()],
            )
            nc.scalar.dma_start(
                out=ht_sb[:].rearrange("p (cg cl) b -> p cg cl b", cg=NCORES),
                in_=d_hxg.ap().rearrange(
                    "(cg p cl b) -> p cg cl b", cg=NCORES, p=128, cl=HBL
                ),
            )

            # ---------------- stage B: fp8 DoubleRow S matmul ----------------
            # psB reserves all 8 banks once psA is closed; bufs=2 gives each
            # pass its own 4 banks so pass 1 accumulates while pass 0 drains.
            psB_cm = tc.tile_pool(name="psB", bufs=2, space="PSUM")
            psB = psB_cm.__enter__()
            pss0 = [psB.tile([128, RBLK], fp32, tag=f"ops{j}", name=f"ps0_{j}")
                    for j in range(HRB)]
            for ph in range(NPASS):
                pss = pss0 if ph == 0 else [
                    psB.tile([128, RBLK], fp32, tag=f"ops{j}", name=f"ps1_{j}")
                    for j in range(HRB)
                ]
                for c2 in range(CCP // 2):
                    st = sstream.tile([128, 2, HRB, 2, RBLK], f8, tag="s")
                    nc.sync.dma_start(out=st[:], in_=d_s.ap()[ph, c2])
                    for q in range(2):
                        cp = c2 * 2 + q
                        for j in range(HRB):
                            nc.tensor.matmul(
                                pss[j][:],
                                lhsT=ht_sb[:, 2 * cp : 2 * cp + 2, :],
                                rhs=st[:, q, j, :, :],
                                start=(cp == 0),
                                stop=(cp == CCP - 1),
                                perf_mode=DROW,
                            )
                for j in range(HRB):
                    rb = ph * HRB + j
                    nc.vector.tensor_add(
                        out=obuf[:, rb, :],
                        in0=pss[j][:],
                        in1=sb_rf[:, rb * RBLK : (rb + 1) * RBLK],
                    )
                nc.scalar.dma_start(
                    out=d_out.ap()[:, ph * HRB * RBLK : (ph + 1) * HRB * RBLK],
                    in_=obuf[:, ph * HRB : (ph + 1) * HRB, :],
                )
            psB_cm.__exit__(None, None, None)

    _split_multiwaits(nc)
    return nc


def _stage_inputs(w, W1, b1, edge_vals, rows, cols, scale, ref):
    """Pure-layout host staging: transposes, COO->dense canonicalization
    (duplicates summed, scipy-style), fp8 packing, and the exact
    ref + 0.5*scale*colsum correction row. No model arithmetic on h."""
    f32 = np.float32
    f16 = np.float16
    e4 = ml_dtypes.float8_e4m3fn
    w = np.asarray(w, dtype=f32)
    W1 = np.asarray(W1, dtype=f32)
    b1 = np.asarray(b1, dtype=f32)
    edge_vals = np.asarray(edge_vals, dtype=f32)
    rows = np.asarray(rows, dtype=np.int64)
    cols = np.asarray(cols, dtype=np.int64)
    scale = np.asarray(scale, dtype=f32)
    ref = np.asarray(ref, dtype=f32)

    def q8(x):
        return np.clip(x, -240.0, 240.0).astype(e4)

    w1t8 = np.ascontiguousarray(q8(W1.T))            # [LATENT, HIDDEN]
    wt8 = np.ascontiguousarray(q8(w.T))              # [LATENT, BATCH]
    b1c = np.ascontiguousarray(0.5 * b1.reshape(HB, 128).T)  # [128, HB]
    HPC = HIDDEN // NCORES                           # hidden units per core

    in_maps = []
    for k in range(NCORES):
        lo, hi = k * RPC, (k + 1) * RPC
        sel = (rows >= lo) & (rows < hi)
        r_k = rows[sel] - lo
        c_k = cols[sel]
        v_k = edge_vals[sel].astype(np.float64)

        # Dense per-core slice S[c, r], duplicate (c, r) entries summed.
        sdense = np.zeros((HIDDEN, RPC), dtype=np.float64)
        np.add.at(sdense, (c_k, r_k), v_k)
        sc = scale[lo:hi].astype(np.float64)
        s8 = q8(0.5 * sdense * sc[None, :])
        # [NPASS, CCP//2, 128, 2, HRB, 2, RBLK]: pass-major 2-ccpair slabs
        # (8KB/partition), DoubleRow K-tile pairs adjacent in dim 5.
        s8 = s8.reshape(CCP // 2, 2, 2, 128, NPASS, HRB, RBLK).transpose(
            4, 0, 3, 1, 5, 2, 6
        )
        s8 = np.ascontiguousarray(s8)

        refp = (ref[lo:hi].astype(np.float64) + 0.5 * sc * sdense.sum(axis=0)).astype(
            f16
        )

        in_maps.append(
            {
                "w1t8": np.ascontiguousarray(w1t8[:, k * HPC : (k + 1) * HPC]),
                "wt8": wt8,
                "b1c": np.ascontiguousarray(b1c[:, k * (HB // NCORES) :][:, : HB // NCORES]),
                "s8": s8,
                "refb": np.ascontiguousarray(
                    np.broadcast_to(refp[None, :], (128, RPC))
                ),
            }
        )
    return in_maps


def kernel(w, W1, b1, edge_vals, rows, cols, scale, ref):
    in_maps = _stage_inputs(w, W1, b1, edge_vals, rows, cols, scale, ref)
    if "nc" not in _NC_CACHE:
        _NC_CACHE["nc"] = _build_nc()
    nc = _NC_CACHE["nc"]
    res = run_bass_kernel_spmd(nc, in_maps, core_ids=list(range(NCORES)))
    out = np.concatenate([r["out"] for r in res.results], axis=1)
    return out.astype(np.float32)


if __name__ == "__main__":
    rng = np.random.default_rng(0)
    nnz = OUT * 32
    ins = {
        "w": rng.standard_normal((BATCH, LATENT), dtype=np.float32),
        "W1": rng.standard_normal((HIDDEN, LATENT), dtype=np.float32),
        "b1": rng.standard_normal(HIDDEN, dtype=np.float32) * 0.01,
        "edge_vals": rng.standard_normal(nnz, dtype=np.float32),
        "rows": np.repeat(np.arange(OUT, dtype=np.int64), 32),
        "cols": rng.integers(0, HIDDEN, nnz).astype(np.int64),
        "scale": rng.random(OUT, dtype=np.float32) + 0.5,
        "ref": rng.standard_normal(OUT, dtype=np.float32),
    }
    out = kernel(**ins)
    print(out.shape, out.dtype)
